# revision 20
# baseline (speedup 1.0000x reference)
"""Trainium2 Bass kernel for nn_AutoEncoding_32641751449755 (vq_codebook).

VQ autoencoder with a BxB adjacency GCN twin-bottleneck, data-parallel over
the batch across 8 NeuronCores (512 rows/core), with AllGathers for the
quantized codes (vn, chunked per row-tile pair), continuous bottleneck (cbn)
and degree vector (dinv).

Activations are kept feature-major ("T layout", feature dim on partitions) so
every matmul contraction has its contraction dim on partitions and biases are
per-partition scalars for the ACT engine. The symmetric normalization of the
adjacency is folded into the GCN matmul (scale cbn rows by dinv_j, scale the
latpre eviction by dinv_i) so nothing waits on the dinv AllGather.

Self-contained: hardcodes shapes; host-side prep = shard + transpose only.
"""

import sys

sys.path.insert(0, "/opt/trn_rl_repo")

import types

import numpy as np

import concourse.bass as bass  # noqa: F401
import concourse.mybir as mybir
import concourse.tile as tile
from concourse import bacc, library_config

F32 = mybir.dt.float32
AF = mybir.ActivationFunctionType
ALU = mybir.AluOpType

N_CORES = 8


def _install_ntff_hook():
    """run_bass_kernel_spmd(trace=True) under axon needs antenv.axon_hooks."""
    if "antenv.axon_hooks" in sys.modules:
        return
    try:
        from trn_agent_boot.trn_boot import _ntff_profile_via_ctypes

        hook = _ntff_profile_via_ctypes("/opt/axon/libaxon_pjrt.so")
    except Exception:
        hook = None
    mod = types.ModuleType("antenv.axon_hooks")
    mod.get_axon_ntff_profile_hook = lambda: hook
    mod.set_axon_ntff_profile_hook = lambda h: None
    sys.modules["antenv.axon_hooks"] = mod


def build(B=4096, Fd=4096, H=1024, L=512, K=8192):
    """Construct the per-core Bass program (SPMD across N_CORES)."""
    BS = B // N_CORES  # rows per core
    BT = BS // 128  # row tiles per core
    FC = Fd // 128
    HC = H // 128
    LC = L // 128
    KC = K // 512  # 512-wide distance chunks
    JC = B // 128  # global row tiles (adjacency columns)
    assert BS <= 512

    # row-tile pairs: pipeline unit for argmin/gather/vn-AllGather
    PAIRS = []
    for p in range(max(1, (BT + 1) // 2)):
        bts = [t for t in (2 * p, 2 * p + 1) if t < BT]
        if bts:
            PAIRS.append((p, bts))

    nc = bacc.Bacc(num_devices=N_CORES)

    # ---- inputs (per core) ----
    xT = nc.dram_tensor("xT", [Fd, BS], F32, kind="ExternalInput")
    w_enc = nc.dram_tensor("w_enc", [Fd, H], F32, kind="ExternalInput")
    b_enc = nc.dram_tensor("b_enc", [H], F32, kind="ExternalInput")
    w_fc1 = nc.dram_tensor("w_fc1", [H, L], F32, kind="ExternalInput")
    b_fc1 = nc.dram_tensor("b_fc1", [L], F32, kind="ExternalInput")
    w_fc2 = nc.dram_tensor("w_fc2", [H, L], F32, kind="ExternalInput")
    b_fc2 = nc.dram_tensor("b_fc2", [L], F32, kind="ExternalInput")
    cT2 = nc.dram_tensor("cT2", [L, K], F32, kind="ExternalInput")  # 2*context.T
    csq = nc.dram_tensor("csq", [K], F32, kind="ExternalInput")  # ||c_k||^2
    ctx = nc.dram_tensor("ctx", [K, L], F32, kind="ExternalInput")
    iota = nc.dram_tensor("iota", [K], F32, kind="ExternalInput")
    ident = nc.dram_tensor("ident", [128, 128], F32, kind="ExternalInput")
    w_gcn = nc.dram_tensor("w_gcn", [L, L], F32, kind="ExternalInput")
    b_gcn = nc.dram_tensor("b_gcn", [L], F32, kind="ExternalInput")
    w_dec1 = nc.dram_tensor("w_dec1", [L, H], F32, kind="ExternalInput")
    b_dec1 = nc.dram_tensor("b_dec1", [H], F32, kind="ExternalInput")
    w_dec2 = nc.dram_tensor("w_dec2", [H, Fd], F32, kind="ExternalInput")
    b_dec2 = nc.dram_tensor("b_dec2", [Fd], F32, kind="ExternalInput")

    # ---- outputs (per core, row shard) ----
    decoded_o = nc.dram_tensor("decoded", [BS, Fd], F32, kind="ExternalOutput")
    bbn_o = nc.dram_tensor("bbn", [BS, L], F32, kind="ExternalOutput")
    cind_o = nc.dram_tensor("context_ind", [BS, K], F32, kind="ExternalOutput")
    feat_o = nc.dram_tensor("feat", [BS, H], F32, kind="ExternalOutput")
    adj_o = nc.dram_tensor("adj", [BS, B], F32, kind="ExternalOutput")

    # ---- internal DRAM (collectives) ----
    vnag_ins, vnag_outs = [], []
    for p, bts in PAIRS:
        pw = 128 * len(bts)
        vnag_ins.append(nc.dram_tensor(f"vnag_in{p}", [L, pw], F32))
        vnag_outs.append(
            nc.dram_tensor(f"vnag_out{p}", [N_CORES * L, pw], F32, addr_space="Shared")
        )
    cbnag_in = nc.dram_tensor("cbnag_in", [BS, L], F32)  # cbn rows
    cbnag_out = nc.dram_tensor("cbnag_out", [B, L], F32, addr_space="Shared")
    idx_d = nc.dram_tensor("idx_d", [BS], mybir.dt.int16)

    rg = [list(range(N_CORES))]

    def bcast_row(dram_t, n):
        """AP reading a [n] dram vector broadcast across 128 partitions."""
        return dram_t.ap().rearrange("(q j) -> q j", q=1).to_broadcast([128, n])

    with tile.TileContext(nc) as tc:
        import contextlib

        est = contextlib.ExitStack()
        with est:
            nc.gpsimd.load_library(library_config.mlp)
            const = est.enter_context(tc.tile_pool(name="const", bufs=1))
            ident_sb = const.tile([128, 128], F32)
            nc.scalar.dma_start(ident_sb[:], ident[:, :])
            benc_sb = const.tile([128, HC], F32)
            nc.scalar.dma_start(benc_sb[:], b_enc.ap().rearrange("(c p) -> p c", p=128))
            bfc1_sb = const.tile([128, LC], F32)
            nc.scalar.dma_start(bfc1_sb[:], b_fc1.ap().rearrange("(c p) -> p c", p=128))
            bfc2_sb = const.tile([128, LC], F32)
            nc.scalar.dma_start(bfc2_sb[:], b_fc2.ap().rearrange("(c p) -> p c", p=128))
            bgcn_sb = const.tile([128, LC], F32)
            nc.scalar.dma_start(bgcn_sb[:], b_gcn.ap().rearrange("(c p) -> p c", p=128))
            bdec1_sb = const.tile([128, HC], F32)
            nc.scalar.dma_start(bdec1_sb[:], b_dec1.ap().rearrange("(c p) -> p c", p=128))
            half_sb = const.tile([128, 1], F32)
            nc.vector.memset(half_sb[:], 0.5)
            eps_sb = const.tile([128, 1], F32)
            nc.vector.memset(eps_sb[:], 1e-8)
            halfb_sb = const.tile([128, 1], F32)
            nc.vector.memset(halfb_sb[:], 0.5 * B + 1e-8)

            # long-lived T-layout activations
            tlay = est.enter_context(tc.tile_pool(name="tlay", bufs=1))
            bbnT = tlay.tile([128, LC, BS], F32)
            vnT = tlay.tile([128, LC, BS], F32)
            latT = tlay.tile([128, LC, BS], F32)
            idx_u32 = tlay.tile([128, BT], mybir.dt.uint32)
            dinv_bt = tlay.tile([128, BT], F32)

            # one-hot staging lives at top level: its trailing writes must
            # not delay the C->E pool handoff
            ohpool = est.enter_context(tc.tile_pool(name="oh", bufs=2))

            # featT spans phases A-E (feat output transposes fill the
            # dinv-AllGather gap after the adjacency phase)
            featp = est.enter_context(tc.tile_pool(name="featT", bufs=1))
            if True:
                featT = featp.tile([128, HC, BS], F32)

                # ---------- phase A: encoder featT = relu(W_enc.T@xT + b) ---
                with (
                    tc.tile_pool(name="xT", bufs=1) as xpool,
                    tc.tile_pool(name="wenc", bufs=3) as wpool,
                    tc.tile_pool(name="ps_a", bufs=1, space="PSUM") as ps_a,
                ):
                    xT_sb = xpool.tile([128, FC, BS], F32)
                    feat_ps = [
                        ps_a.tile([128, BS], F32, name=f"fps{h}", tag=f"fps{h}")
                        for h in range(HC)
                    ]
                    for f in range(FC):
                        nc.sync.dma_start(
                            xT_sb[:, f, :], xT[f * 128 : (f + 1) * 128, :]
                        )
                        wband = wpool.tile([128, H], F32, tag="wband")
                        nc.sync.dma_start(wband[:], w_enc[f * 128 : (f + 1) * 128, :])
                        for h in range(HC):
                            nc.tensor.matmul(
                                feat_ps[h][:],
                                wband[:, h * 128 : (h + 1) * 128],
                                xT_sb[:, f, :],
                                start=(f == 0),
                                stop=(f == FC - 1),
                            )
                    for h in range(HC):
                        nc.scalar.activation(
                            out=featT[:, h, :],
                            in_=feat_ps[h][:],
                            func=AF.Relu,
                            bias=benc_sb[:, h : h + 1],
                            scale=1.0,
                        )

                # ---------- phase B: bbnT/cbnT + cbn AllGather --------------
                with tc.tile_pool(name="cbns", bufs=1) as cbns:
                    cbnT_stage = cbns.tile([128, LC, BS], F32)
                    b_inner = contextlib.ExitStack()
                    wfc_pool = b_inner.enter_context(
                        tc.tile_pool(name="wfc", bufs=2)
                    )
                    ps_b = b_inner.enter_context(
                        tc.tile_pool(name="ps_b", bufs=1, space="PSUM")
                    )
                    bbn_ps = [
                        ps_b.tile([128, BS], F32, name=f"bps{lc}", tag=f"bps{lc}")
                        for lc in range(LC)
                    ]
                    cbn_ps = [
                        ps_b.tile([128, BS], F32, name=f"cps{lc}", tag=f"cps{lc}")
                        for lc in range(LC)
                    ]
                    for h in range(HC):
                        w1b = wfc_pool.tile([128, L], F32, tag="w1")
                        w2b = wfc_pool.tile([128, L], F32, tag="w2")
                        nc.sync.dma_start(w1b[:], w_fc1[h * 128 : (h + 1) * 128, :])
                        nc.sync.dma_start(w2b[:], w_fc2[h * 128 : (h + 1) * 128, :])
                        for lc in range(LC):
                            nc.tensor.matmul(
                                bbn_ps[lc][:],
                                w1b[:, lc * 128 : (lc + 1) * 128],
                                featT[:, h, :],
                                start=(h == 0),
                                stop=(h == HC - 1),
                            )
                            nc.tensor.matmul(
                                cbn_ps[lc][:],
                                w2b[:, lc * 128 : (lc + 1) * 128],
                                featT[:, h, :],
                                start=(h == 0),
                                stop=(h == HC - 1),
                            )
                    for lc in range(LC):
                        nc.scalar.activation(
                            out=bbnT[:, lc, :],
                            in_=bbn_ps[lc][:],
                            func=AF.Identity,
                            bias=bfc1_sb[:, lc : lc + 1],
                            scale=1.0,
                        )
                        nc.scalar.activation(
                            out=cbnT_stage[:, lc, :],
                            in_=cbn_ps[lc][:],
                            func=AF.Identity,
                            bias=bfc2_sb[:, lc : lc + 1],
                            scale=1.0,
                        )
                    b_inner.close()
                    # cbn rows -> DRAM -> AllGather (early; hidden by phase C)
                    ps_tr = b_inner.enter_context(
                        tc.tile_pool(name="ps_tr", bufs=4, space="PSUM")
                    )
                    cbnr = cbns.tile([128, BT, L], F32)
                    for bt in range(BT):
                        for lc in range(LC):
                            tp = ps_tr.tile([128, 128], F32, tag="tp")
                            nc.tensor.transpose(
                                tp[:],
                                cbnT_stage[:, lc, bt * 128 : (bt + 1) * 128],
                                ident_sb[:],
                            )
                            nc.vector.tensor_copy(
                                cbnr[:, bt, lc * 128 : (lc + 1) * 128], tp[:]
                            )
                    nc.sync.dma_start(
                        cbnag_in.ap().rearrange("(t p) l -> p t l", p=128), cbnr[:]
                    )
                    nc.gpsimd.collective_compute(
                        "AllGather",
                        ALU.bypass,
                        replica_groups=rg,
                        ins=[cbnag_in.ap().opt()],
                        outs=[cbnag_out.ap().opt()],
                    )
                    b_inner.close()

                # ---------- phases C+D: distance/argmin/one-hot/gather/vn ---
                with (
                    tc.tile_pool(name="gat", bufs=1) as gatp,
                    tc.tile_pool(name="small_c", bufs=2 * BT) as smallp,
                    tc.tile_pool(name="csqp", bufs=1) as csqp,
                    tc.tile_pool(name="c2", bufs=2) as c2pool,
                    tc.tile_pool(name="sbuf_s", bufs=2) as spool,
                    tc.tile_pool(name="ps_c", bufs=6, space="PSUM") as ps_c,
                    tc.tile_pool(name="ps_d", bufs=2, space="PSUM") as ps_d,
                ):
                    csq_bc = csqp.tile([128, K], F32)
                    nc.scalar.dma_start(csq_bc[:], bcast_row(csq, K))

                    for p, bts in PAIRS:
                        pw = 128 * len(bts)
                        s_bufs = {
                            t: spool.tile([128, K], F32, name=f"sbuf{t}", tag="sbuf")
                            for t in bts
                        }
                        for kc in range(KC):
                            c2b = c2pool.tile([128, LC, 512], F32, tag="c2b")
                            nc.sync.dma_start(
                                c2b[:],
                                cT2.ap()[:, kc * 512 : (kc + 1) * 512].rearrange(
                                    "(c p) k -> p c k", p=128
                                ),
                            )
                            for t in bts:
                                s_ps = ps_c.tile([128, 512], F32, tag="sps")
                                for lc in range(LC):
                                    nc.tensor.matmul(
                                        s_ps[:],
                                        bbnT[:, lc, t * 128 : (t + 1) * 128],
                                        c2b[:, lc, :],
                                        start=(lc == 0),
                                        stop=(lc == LC - 1),
                                    )
                                # score = 2*bbn.c - ||c||^2 (argmax==argmin d)
                                nc.vector.scalar_tensor_tensor(
                                    out=s_bufs[t][:, kc * 512 : (kc + 1) * 512],
                                    in0=s_ps[:],
                                    scalar=1.0,
                                    in1=csq_bc[:, kc * 512 : (kc + 1) * 512],
                                    op0=ALU.mult,
                                    op1=ALU.subtract,
                                )
                        # argmin (one-hot deferred until after the vn AG)
                        idx_fs = {}
                        for t in bts:
                            mx8 = smallp.tile([128, 8], F32, tag="mx8")
                            ix8 = smallp.tile([128, 8], mybir.dt.uint32, tag="ix8")
                            nc.vector.max(mx8[:], s_bufs[t][:])
                            nc.vector.max_index(ix8[:], mx8[:], s_bufs[t][:])
                            nc.vector.tensor_copy(idx_u32[:, t : t + 1], ix8[:, 0:1])
                            idx_f = smallp.tile(
                                [128, 1], F32, name=f"idxf{t}", tag=f"idxf{t % 2}"
                            )
                            nc.vector.tensor_copy(idx_f[:], ix8[:, 0:1])
                            idx_fs[t] = idx_f

                        # per-tile gather + normalize + transpose -> vnT
                        off = bts[0] * 128
                        for t in bts:
                            toff = t * 128
                            idx16 = smallp.tile(
                                [128, 1], mybir.dt.int16, name=f"idx16_{t}",
                                tag=f"idx16_{t % 2}",
                            )
                            nc.vector.tensor_copy(idx16[:], idx_u32[:, t : t + 1])
                            nc.gpsimd.dma_start(
                                idx_d.ap()[toff : toff + 128].rearrange(
                                    "(t p) -> p t", p=128
                                ),
                                idx16[:],
                            )
                            idx_w = gatp.tile(
                                [128, 8], mybir.dt.int16, name=f"idxw{t}",
                                tag=f"idxw{t % 2}",
                            )
                            for r in range(8):
                                nc.gpsimd.dma_start(
                                    idx_w[16 * r : 16 * (r + 1), :],
                                    idx_d.ap()[toff : toff + 128].rearrange(
                                        "(s q) -> q s", q=16
                                    ),
                                )
                            quant = gatp.tile(
                                [128, 1, L], F32, name=f"qt{t}", tag=f"qt{t % 2}"
                            )
                            nc.gpsimd.dma_gather(
                                out_ap=quant[:],
                                in_ap=ctx.ap(),
                                idxs_ap=idx_w[:],
                                num_idxs=128,
                                num_idxs_reg=128,
                                elem_size=L,
                            )
                            sqtmp = gatp.tile(
                                [128, L], F32, name=f"sq{t}", tag=f"sq{t % 2}"
                            )
                            ss = smallp.tile([128, 1], F32, tag="ssn")
                            nc.scalar.activation(
                                out=sqtmp[:],
                                in_=quant[:, 0, :],
                                func=AF.Square,
                                accum_out=ss[:],
                            )
                            nc.scalar.activation(
                                out=ss[:], in_=ss[:], func=AF.Sqrt, bias=eps_sb[:, 0:1]
                            )
                            nc.vector.reciprocal(out=ss[:], in_=ss[:])
                            nc.vector.tensor_scalar(
                                out=quant[:, 0, :],
                                in0=quant[:, 0, :],
                                scalar1=ss[:],
                                scalar2=None,
                                op0=ALU.mult,
                            )
                            for lc in range(LC):
                                tp = ps_d.tile([128, 128], F32, tag="tp")
                                nc.tensor.transpose(
                                    tp[:],
                                    quant[:, 0, lc * 128 : (lc + 1) * 128],
                                    ident_sb[:],
                                )
                                nc.vector.tensor_copy(
                                    vnT[:, lc, t * 128 : (t + 1) * 128], tp[:]
                                )
                            # one-hot context_ind rows (off critical path)
                            KH = K // 2
                            for hh in range(2):
                                oh = ohpool.tile(
                                    [128, KH], F32, name=f"oh{t}_{hh}", tag="oh"
                                )
                                nc.scalar.dma_start(
                                    oh[:],
                                    iota.ap()[hh * KH : (hh + 1) * KH]
                                    .rearrange("(q j) -> q j", q=1)
                                    .to_broadcast([128, KH]),
                                )
                                nc.vector.tensor_scalar(
                                    out=oh[:],
                                    in0=oh[:],
                                    scalar1=idx_fs[t][:],
                                    scalar2=None,
                                    op0=ALU.is_equal,
                                )
                                nc.scalar.dma_start(
                                    cind_o[
                                        t * 128 : (t + 1) * 128,
                                        hh * KH : (hh + 1) * KH,
                                    ],
                                    oh[:],
                                )
                        nc.gpsimd.dma_start(
                            vnag_ins[p]
                            .ap()
                            .rearrange("(c q) b -> q c b", q=128),
                            vnT[:, :, off : off + pw],
                        )
                        nc.gpsimd.collective_compute(
                            "AllGather",
                            ALU.bypass,
                            replica_groups=rg,
                            ins=[vnag_ins[p].ap().opt()],
                            outs=[vnag_outs[p].ap().opt()],
                        )


            # ---------- phases E+F: adjacency + GCN -------------------------
            with tc.tile_pool(name="adj", bufs=1) as adjp:
                adj_sb = adjp.tile([128, BT, B], F32)
                dinv_sb = tlay.tile([128, JC], F32)
                with (
                    tc.tile_pool(name="small_e", bufs=2 * BT) as small_e,
                    tc.tile_pool(name="ps_e", bufs=3, space="PSUM") as ps_e,
                    tc.tile_pool(name="ps_dv", bufs=2, space="PSUM") as ps_dv,
                ):
                    NP = len(PAIRS)
                    rss = [
                        small_e.tile(
                            [128, N_CORES * NP], F32, name=f"rs{bt}", tag=f"rs{bt}"
                        )
                        for bt in range(BT)
                    ]
                    # partial row-sums of vn (for the closed-form column sums)
                    spart = small_e.tile([128, LC, NP * N_CORES], F32, name="spart")
                    s_col = small_e.tile([128, LC], F32, name="s_col")
                    vnag_sbs = {}
                    vnag_stack = contextlib.ExitStack()
                    for p, bts in PAIRS:
                        pw = 128 * len(bts)
                        off = bts[0] * 128
                        vnagp = vnag_stack.enter_context(
                            tc.tile_pool(name=f"vnag{p}", bufs=1)
                        )
                        vsb = vnagp.tile(
                            [128, N_CORES * LC, pw],
                            F32,
                            name=f"vnag{p}",
                            tag=f"vg{p}",
                        )
                        nc.scalar.dma_start(
                            vsb[:],
                            vnag_outs[p].ap().rearrange("(c q) b -> q c b", q=128),
                        )
                        vnag_sbs[p] = vsb
                        for c in range(N_CORES):
                            for lc in range(LC):
                                nc.vector.reduce_sum(
                                    spart[:, lc, p * N_CORES + c : p * N_CORES + c + 1],
                                    vsb[:, c * LC + lc, :],
                                    axis=mybir.AxisListType.X,
                                )
                        for bt in range(BT):
                            for c in range(N_CORES):
                                a_ps = ps_e.tile([128, pw], F32, tag="aps")
                                for lc in range(LC):
                                    nc.tensor.matmul(
                                        a_ps[:],
                                        vnT[:, lc, bt * 128 : (bt + 1) * 128],
                                        vnag_sbs[p][:, c * LC + lc, :],
                                        start=(lc == 0),
                                        stop=(lc == LC - 1),
                                    )
                                nc.scalar.activation(
                                    out=adj_sb[
                                        :, bt, c * BS + off : c * BS + off + pw
                                    ],
                                    in_=a_ps[:],
                                    func=AF.Identity,
                                    bias=half_sb[:, 0:1],
                                    scale=0.5,
                                    accum_out=rss[bt][
                                        :, p * N_CORES + c : p * N_CORES + c + 1
                                    ],
                                )
                    # dinv for our rows (exact row sums, matches reference)
                    for bt in range(BT):
                        rsum = small_e.tile([128, 1], F32, tag="rsum")
                        nc.vector.reduce_sum(
                            rsum[:], rss[bt][:], axis=mybir.AxisListType.X
                        )
                        nc.scalar.activation(
                            out=rsum[:], in_=rsum[:], func=AF.Sqrt, bias=eps_sb[:, 0:1]
                        )
                        nc.vector.reciprocal(out=rsum[:], in_=rsum[:])
                        nc.vector.tensor_copy(dinv_bt[:, bt : bt + 1], rsum[:])
                        nc.scalar.dma_start(
                            adj_o[bt * 128 : (bt + 1) * 128, :], adj_sb[:, bt, :]
                        )
                    # dinv for all columns, closed form:
                    # colsum_j = 0.5*(vn_j . S) + 0.5*B  (S = sum of all vn rows)
                    nc.vector.reduce_sum(
                        s_col[:], spart[:], axis=mybir.AxisListType.X
                    )
                    for m in range(JC):
                        c, t = m // BT, m % BT
                        p, o2 = t // 2, (t % 2) * 128
                        d_ps = ps_dv.tile([128, 1], F32, tag="dv")
                        for lc in range(LC):
                            nc.tensor.matmul(
                                d_ps[:],
                                vnag_sbs[p][:, c * LC + lc, o2 : o2 + 128],
                                s_col[:, lc : lc + 1],
                                start=(lc == 0),
                                stop=(lc == LC - 1),
                            )
                        nc.scalar.activation(
                            out=dinv_sb[:, m : m + 1],
                            in_=d_ps[:],
                            func=AF.Sqrt,
                            bias=halfb_sb[:, 0:1],
                            scale=0.5,
                        )
                    nc.vector.reciprocal(out=dinv_sb[:], in_=dinv_sb[:])
                    vnag_stack.close()
                    # bbn/feat row-major outputs: fills the dinv-AllGather gap
                    with tc.tile_pool(name="stg", bufs=1) as stg:
                        for bt in range(BT):
                            bstg = stg.tile([128, L], F32, tag="bstg")
                            for lc in range(LC):
                                tp = ps_e.tile([128, 128], F32, tag="tp")
                                nc.tensor.transpose(
                                    tp[:],
                                    bbnT[:, lc, bt * 128 : (bt + 1) * 128],
                                    ident_sb[:],
                                )
                                nc.vector.tensor_copy(
                                    bstg[:, lc * 128 : (lc + 1) * 128], tp[:]
                                )
                            nc.scalar.dma_start(
                                bbn_o[bt * 128 : (bt + 1) * 128, :], bstg[:]
                            )
                        for bt in range(BT):
                            fstg = stg.tile([128, H], F32, tag="fstg")
                            for h in range(HC):
                                tp = ps_e.tile([128, 128], F32, tag="tp")
                                nc.tensor.transpose(
                                    tp[:],
                                    featT[:, h, bt * 128 : (bt + 1) * 128],
                                    ident_sb[:],
                                )
                                nc.vector.tensor_copy(
                                    fstg[:, h * 128 : (h + 1) * 128], tp[:]
                                )
                            nc.scalar.dma_start(
                                feat_o[bt * 128 : (bt + 1) * 128, :], fstg[:]
                            )

                # ---- GCN layer 1 (normalization folded):
                # latpre = dinv_i * sum_j adj[i,j] * (dinv_j * cbn[j])
                with (
                    tc.tile_pool(name="cbnagp", bufs=4) as cbnagp,
                    tc.tile_pool(name="natp", bufs=32) as natp,
                    tc.tile_pool(name="latp", bufs=1) as latp,
                ):
                    f_inner = contextlib.ExitStack()
                    ps_lp = f_inner.enter_context(
                        tc.tile_pool(name="ps_lp", bufs=1, space="PSUM")
                    )
                    ps_tp2 = f_inner.enter_context(
                        tc.tile_pool(name="ps_tp2", bufs=4, space="PSUM")
                    )
                    latpre = latp.tile([128, BT, L], F32)
                    lp_ps = [
                        ps_lp.tile([128, L], F32, name=f"lpps{bt}", tag=f"lpps{bt}")
                        for bt in range(BT)
                    ]
                    for m in range(JC):
                        cb = cbnagp.tile([128, L], F32, tag="cb")
                        nc.sync.dma_start(cb[:], cbnag_out[m * 128 : (m + 1) * 128, :])
                        nc.vector.tensor_scalar(
                            out=cb[:],
                            in0=cb[:],
                            scalar1=dinv_sb[:, m : m + 1],
                            scalar2=None,
                            op0=ALU.mult,
                        )
                        naTs = []
                        for bt in range(BT):
                            tp = ps_tp2.tile([128, 128], F32, tag="tp")
                            nc.tensor.transpose(
                                tp[:],
                                adj_sb[:, bt, m * 128 : (m + 1) * 128],
                                ident_sb[:],
                            )
                            naT = natp.tile([128, 128], F32, tag="naT")
                            nc.vector.tensor_copy(naT[:], tp[:])
                            naTs.append(naT)
                        for bt in range(BT):
                            nc.tensor.matmul(
                                lp_ps[bt][:],
                                naTs[bt][:],
                                cb[:],
                                start=(m == 0),
                                stop=(m == JC - 1),
                            )
                    for bt in range(BT):
                        nc.scalar.activation(
                            out=latpre[:, bt, :],
                            in_=lp_ps[bt][:],
                            func=AF.Copy,
                            scale=dinv_bt[:, bt : bt + 1],
                        )

                    # latpreT + zT = W_gcn.T @ latpreT, sigmoid -> latT
                    latpreT = latp.tile([128, LC, BS], F32)
                    for bt in range(BT):
                        for lc in range(LC):
                            tp = ps_tp2.tile([128, 128], F32, tag="tp")
                            nc.tensor.transpose(
                                tp[:],
                                latpre[:, bt, lc * 128 : (lc + 1) * 128],
                                ident_sb[:],
                            )
                            nc.vector.tensor_copy(
                                latpreT[:, lc, bt * 128 : (bt + 1) * 128], tp[:]
                            )
                    f_inner.close()
                    with (
                        tc.tile_pool(name="wgcnp", bufs=1) as wgcnp,
                        tc.tile_pool(name="ps_z", bufs=2, space="PSUM") as ps_z,
                    ):
                        wgcn_sb = wgcnp.tile([128, LC, L], F32)
                        nc.sync.dma_start(
                            wgcn_sb[:], w_gcn.ap().rearrange("(c p) l -> p c l", p=128)
                        )
                        for gc in range(LC):
                            z_ps = ps_z.tile([128, BS], F32, tag="z")
                            for lc in range(LC):
                                nc.tensor.matmul(
                                    z_ps[:],
                                    wgcn_sb[:, lc, gc * 128 : (gc + 1) * 128],
                                    latpreT[:, lc, :],
                                    start=(lc == 0),
                                    stop=(lc == LC - 1),
                                )
                            nc.scalar.activation(
                                out=latT[:, gc, :],
                                in_=z_ps[:],
                                func=AF.Sigmoid,
                                bias=bgcn_sb[:, gc : gc + 1],
                                scale=1.0,
                            )

            # ---------- phase G: decoder ------------------------------------
            with (
                tc.tile_pool(name="wdecp", bufs=2) as wdecp,
                tc.tile_pool(name="d1p", bufs=1) as d1p,
                tc.tile_pool(name="b2p", bufs=1) as b2p,
                tc.tile_pool(name="dstage", bufs=3) as dstage,
                tc.tile_pool(name="ps_d1", bufs=2, space="PSUM") as ps_d1,
                tc.tile_pool(name="ps_o", bufs=4, space="PSUM") as ps_o,
            ):
                wdec1_sb = wdecp.tile([128, LC, H], F32, tag="w1")
                nc.sync.dma_start(
                    wdec1_sb[:], w_dec1.ap().rearrange("(c p) h -> p c h", p=128)
                )
                d1T = d1p.tile([128, HC, BS], F32)
                for hc in range(HC):
                    d_ps = ps_d1.tile([128, BS], F32, tag="d1")
                    for gc in range(LC):
                        nc.tensor.matmul(
                            d_ps[:],
                            wdec1_sb[:, gc, hc * 128 : (hc + 1) * 128],
                            latT[:, gc, :],
                            start=(gc == 0),
                            stop=(gc == LC - 1),
                        )
                    nc.scalar.activation(
                        out=d1T[:, hc, :],
                        in_=d_ps[:],
                        func=AF.Relu,
                        bias=bdec1_sb[:, hc : hc + 1],
                        scale=1.0,
                    )
                b2_bc = b2p.tile([128, Fd], F32)
                nc.sync.dma_start(b2_bc[:], bcast_row(b_dec2, Fd))
                for fc in range(Fd // 512):
                    w2blk = wdecp.tile([128, HC, 512], F32, tag="w2")
                    nc.sync.dma_start(
                        w2blk[:],
                        w_dec2.ap()[:, fc * 512 : (fc + 1) * 512].rearrange(
                            "(c p) f -> p c f", p=128
                        ),
                    )
                    for bt in range(BT):
                        o_ps = ps_o.tile([128, 512], F32, tag="ops")
                        for hc in range(HC):
                            nc.tensor.matmul(
                                o_ps[:],
                                d1T[:, hc, bt * 128 : (bt + 1) * 128],
                                w2blk[:, hc, :],
                                start=(hc == 0),
                                stop=(hc == HC - 1),
                            )
                        dec_sb = dstage.tile([128, 512], F32, tag="dec")
                        nc.vector.tensor_tensor(
                            out=dec_sb[:],
                            in0=o_ps[:],
                            in1=b2_bc[:, fc * 512 : (fc + 1) * 512],
                            op=ALU.add,
                        )
                        nc.sync.dma_start(
                            decoded_o[
                                bt * 128 : (bt + 1) * 128, fc * 512 : (fc + 1) * 512
                            ],
                            dec_sb[:],
                        )

    nc.compile()
    return nc


def prep_inputs(inputs, B=4096, Fd=4096, H=1024, L=512, K=8192):
    """Host-side shard + transpose. Returns per-core input maps."""
    BS = B // N_CORES
    x = np.asarray(inputs["x"], np.float32)
    context = np.ascontiguousarray(np.asarray(inputs["context"], np.float32))
    cT2 = np.ascontiguousarray(context.T * 2.0)
    csq = np.sum(context * context, axis=-1).astype(np.float32)
    iota = np.arange(K, dtype=np.float32)
    ident = np.eye(128, dtype=np.float32)
    shared = {
        "w_enc": np.ascontiguousarray(np.asarray(inputs["W_enc"], np.float32)),
        "b_enc": np.asarray(inputs["b_enc"], np.float32),
        "w_fc1": np.ascontiguousarray(np.asarray(inputs["W_fc1"], np.float32)),
        "b_fc1": np.asarray(inputs["b_fc1"], np.float32),
        "w_fc2": np.ascontiguousarray(np.asarray(inputs["W_fc2"], np.float32)),
        "b_fc2": np.asarray(inputs["b_fc2"], np.float32),
        "cT2": cT2,
        "csq": csq,
        "ctx": context,
        "iota": iota,
        "ident": ident,
        "w_gcn": np.ascontiguousarray(np.asarray(inputs["W_gcn"], np.float32)),
        "b_gcn": np.asarray(inputs["b_gcn"], np.float32),
        "w_dec1": np.ascontiguousarray(np.asarray(inputs["W_dec1"], np.float32)),
        "b_dec1": np.asarray(inputs["b_dec1"], np.float32),
        "w_dec2": np.ascontiguousarray(np.asarray(inputs["W_dec2"], np.float32)),
        "b_dec2": np.asarray(inputs["b_dec2"], np.float32),
    }
    in_maps = []
    for c in range(N_CORES):
        m = dict(shared)
        m["xT"] = np.ascontiguousarray(x[c * BS : (c + 1) * BS, :].T)
        in_maps.append(m)
    return in_maps


def collect_outputs(results):
    decoded = np.concatenate([r["decoded"] for r in results], axis=0)
    bbn = np.concatenate([r["bbn"] for r in results], axis=0)
    cind = np.concatenate([r["context_ind"] for r in results], axis=0)
    feat = np.concatenate([r["feat"] for r in results], axis=0)
    adj = np.concatenate([r["adj"] for r in results], axis=0)
    return decoded, bbn, cind, feat, adj


_NC_CACHE = {}


def kernel(**inputs):
    B, Fd, H, L, K = 4096, 4096, 1024, 512, 8192
    _install_ntff_hook()
    from concourse.bass_utils import run_bass_kernel_spmd

    key = (B, Fd, H, L, K)
    if key not in _NC_CACHE:
        _NC_CACHE[key] = build(B, Fd, H, L, K)
    nc = _NC_CACHE[key]
    in_maps = prep_inputs(inputs, B, Fd, H, L, K)
    res = run_bass_kernel_spmd(nc, in_maps, core_ids=list(range(N_CORES)))
    return collect_outputs(res.results)


# revision 24
# speedup vs baseline: 1.4049x; 1.4049x over previous
"""Trainium2 Bass kernel for nn_AutoEncoding_32641751449755 (vq_codebook).

VQ autoencoder with a BxB adjacency GCN twin-bottleneck, data-parallel over
the batch across 8 NeuronCores (512 rows/core), with AllGathers for the
quantized codes (vn, chunked per row-tile pair), continuous bottleneck (cbn)
and degree vector (dinv).

Activations are kept feature-major ("T layout", feature dim on partitions) so
every matmul contraction has its contraction dim on partitions and biases are
per-partition scalars for the ACT engine. The symmetric normalization of the
adjacency is folded into the GCN matmul (scale cbn rows by dinv_j, scale the
latpre eviction by dinv_i) so nothing waits on the dinv AllGather.

Self-contained: hardcodes shapes; host-side prep = shard + transpose only.
"""

import sys

sys.path.insert(0, "/opt/trn_rl_repo")

import types

import numpy as np

import concourse.bass as bass  # noqa: F401
import concourse.mybir as mybir
import concourse.tile as tile
from concourse import bacc, library_config

F32 = mybir.dt.float32
BF16 = mybir.dt.bfloat16
AF = mybir.ActivationFunctionType

# bf16 backend for the post-argmin stages (adjacency gram, GCN, decoder).
# The encoder/fc1/distance/argmin path stays pure fp32 so context_ind is exact.
BF16_BACKEND = True
ALU = mybir.AluOpType

N_CORES = 8


def _install_ntff_hook():
    """run_bass_kernel_spmd(trace=True) under axon needs antenv.axon_hooks."""
    if "antenv.axon_hooks" in sys.modules:
        return
    try:
        from trn_agent_boot.trn_boot import _ntff_profile_via_ctypes

        hook = _ntff_profile_via_ctypes("/opt/axon/libaxon_pjrt.so")
    except Exception:
        hook = None
    mod = types.ModuleType("antenv.axon_hooks")
    mod.get_axon_ntff_profile_hook = lambda: hook
    mod.set_axon_ntff_profile_hook = lambda h: None
    sys.modules["antenv.axon_hooks"] = mod


def build(B=4096, Fd=4096, H=1024, L=512, K=8192, bf16=BF16_BACKEND):
    """Construct the per-core Bass program (SPMD across N_CORES)."""
    BS = B // N_CORES  # rows per core
    BT = BS // 128  # row tiles per core
    FC = Fd // 128
    HC = H // 128
    LC = L // 128
    KC = K // 512  # 512-wide distance chunks
    JC = B // 128  # global row tiles (adjacency columns)
    assert BS <= 512

    # row-tile pairs: pipeline unit for argmin/gather/vn-AllGather
    PAIRS = []
    for p in range(max(1, (BT + 1) // 2)):
        bts = [t for t in (2 * p, 2 * p + 1) if t < BT]
        if bts:
            PAIRS.append((p, bts))

    DTB = BF16 if bf16 else F32
    nc = bacc.Bacc(num_devices=N_CORES)

    # ---- inputs (per core) ----
    xT = nc.dram_tensor("xT", [Fd, BS], F32, kind="ExternalInput")
    w_enc = nc.dram_tensor("w_enc", [Fd, H], F32, kind="ExternalInput")
    b_enc = nc.dram_tensor("b_enc", [H], F32, kind="ExternalInput")
    w_fc1 = nc.dram_tensor("w_fc1", [H, L], F32, kind="ExternalInput")
    b_fc1 = nc.dram_tensor("b_fc1", [L], F32, kind="ExternalInput")
    w_fc2 = nc.dram_tensor("w_fc2", [H, L], F32, kind="ExternalInput")
    b_fc2 = nc.dram_tensor("b_fc2", [L], F32, kind="ExternalInput")
    cT2 = nc.dram_tensor("cT2", [L, K], F32, kind="ExternalInput")  # 2*context.T
    csq = nc.dram_tensor("csq", [K], F32, kind="ExternalInput")  # ||c_k||^2
    ctx = nc.dram_tensor("ctx", [K, L], F32, kind="ExternalInput")
    iota = nc.dram_tensor("iota", [K], F32, kind="ExternalInput")
    ident = nc.dram_tensor("ident", [128, 128], F32, kind="ExternalInput")
    w_gcn = nc.dram_tensor("w_gcn", [L, L], DTB, kind="ExternalInput")
    b_gcn = nc.dram_tensor("b_gcn", [L], F32, kind="ExternalInput")
    w_dec1 = nc.dram_tensor("w_dec1", [L, H], DTB, kind="ExternalInput")
    b_dec1 = nc.dram_tensor("b_dec1", [H], F32, kind="ExternalInput")
    w_dec2 = nc.dram_tensor("w_dec2", [H, Fd], DTB, kind="ExternalInput")
    b_dec2 = nc.dram_tensor("b_dec2", [Fd], F32, kind="ExternalInput")

    # ---- outputs (per core, row shard) ----
    decoded_o = nc.dram_tensor("decoded", [BS, Fd], F32, kind="ExternalOutput")
    bbn_o = nc.dram_tensor("bbn", [BS, L], F32, kind="ExternalOutput")
    cind_o = nc.dram_tensor("context_ind", [BS, K], F32, kind="ExternalOutput")
    feat_o = nc.dram_tensor("feat", [BS, H], F32, kind="ExternalOutput")
    adj_o = nc.dram_tensor("adj", [BS, B], F32, kind="ExternalOutput")

    # ---- internal DRAM (collectives) ----
    vnag_ins, vnag_outs = [], []
    for p, bts in PAIRS:
        pw = 128 * len(bts)
        vnag_ins.append(nc.dram_tensor(f"vnag_in{p}", [L, pw], DTB))
        vnag_outs.append(
            nc.dram_tensor(f"vnag_out{p}", [N_CORES * L, pw], DTB, addr_space="Shared")
        )
    cbnag_in = nc.dram_tensor("cbnag_in", [BS, L], DTB)  # cbn rows
    cbnag_out = nc.dram_tensor("cbnag_out", [B, L], DTB, addr_space="Shared")
    idx_d = nc.dram_tensor("idx_d", [BS], mybir.dt.int16)

    rg = [list(range(N_CORES))]

    def bcast_row(dram_t, n):
        """AP reading a [n] dram vector broadcast across 128 partitions."""
        return dram_t.ap().rearrange("(q j) -> q j", q=1).to_broadcast([128, n])

    with tile.TileContext(nc) as tc:
        import contextlib

        est = contextlib.ExitStack()
        with est:
            nc.gpsimd.load_library(library_config.mlp)
            const = est.enter_context(tc.tile_pool(name="const", bufs=1))
            ident_sb = const.tile([128, 128], F32)
            nc.scalar.dma_start(ident_sb[:], ident[:, :])
            benc_sb = const.tile([128, HC], F32)
            nc.scalar.dma_start(benc_sb[:], b_enc.ap().rearrange("(c p) -> p c", p=128))
            bfc1_sb = const.tile([128, LC], F32)
            nc.scalar.dma_start(bfc1_sb[:], b_fc1.ap().rearrange("(c p) -> p c", p=128))
            bfc2_sb = const.tile([128, LC], F32)
            nc.scalar.dma_start(bfc2_sb[:], b_fc2.ap().rearrange("(c p) -> p c", p=128))
            bgcn_sb = const.tile([128, LC], F32)
            nc.scalar.dma_start(bgcn_sb[:], b_gcn.ap().rearrange("(c p) -> p c", p=128))
            bdec1_sb = const.tile([128, HC], F32)
            nc.scalar.dma_start(bdec1_sb[:], b_dec1.ap().rearrange("(c p) -> p c", p=128))
            half_sb = const.tile([128, 1], F32)
            nc.vector.memset(half_sb[:], 0.5)
            eps_sb = const.tile([128, 1], F32)
            nc.vector.memset(eps_sb[:], 1e-8)
            halfb_sb = const.tile([128, 1], F32)
            nc.vector.memset(halfb_sb[:], 0.5 * B + 1e-8)

            # long-lived T-layout activations
            tlay = est.enter_context(tc.tile_pool(name="tlay", bufs=1))
            bbnT = tlay.tile([128, LC, BS], F32)
            vnT = tlay.tile([128, LC, BS], DTB)
            latT = tlay.tile([128, LC, BS], DTB)
            idx_u32 = tlay.tile([128, BT], mybir.dt.uint32)
            dinv_bt = tlay.tile([128, BT], F32)

            # one-hot staging lives at top level: its trailing writes must
            # not delay the C->E pool handoff
            ohpool = est.enter_context(tc.tile_pool(name="oh", bufs=2))

            # featT spans phases A-E (feat output transposes fill the
            # dinv-AllGather gap after the adjacency phase)
            featp = est.enter_context(tc.tile_pool(name="featT", bufs=1))
            if True:
                featT = featp.tile([128, HC, BS], F32)

                # ---------- phase A: encoder featT = relu(W_enc.T@xT + b) ---
                with (
                    tc.tile_pool(name="xT", bufs=1) as xpool,
                    tc.tile_pool(name="wenc", bufs=3) as wpool,
                    tc.tile_pool(name="ps_a", bufs=1, space="PSUM") as ps_a,
                ):
                    xT_sb = xpool.tile([128, FC, BS], F32)
                    feat_ps = [
                        ps_a.tile([128, BS], F32, name=f"fps{h}", tag=f"fps{h}")
                        for h in range(HC)
                    ]
                    for f in range(FC):
                        nc.sync.dma_start(
                            xT_sb[:, f, :], xT[f * 128 : (f + 1) * 128, :]
                        )
                        wband = wpool.tile([128, H], F32, tag="wband")
                        nc.sync.dma_start(wband[:], w_enc[f * 128 : (f + 1) * 128, :])
                        for h in range(HC):
                            nc.tensor.matmul(
                                feat_ps[h][:],
                                wband[:, h * 128 : (h + 1) * 128],
                                xT_sb[:, f, :],
                                start=(f == 0),
                                stop=(f == FC - 1),
                            )
                    for h in range(HC):
                        nc.scalar.activation(
                            out=featT[:, h, :],
                            in_=feat_ps[h][:],
                            func=AF.Relu,
                            bias=benc_sb[:, h : h + 1],
                            scale=1.0,
                        )

                # ---------- phase B: bbnT/cbnT + cbn AllGather --------------
                with tc.tile_pool(name="cbns", bufs=1) as cbns:
                    cbnT_stage = cbns.tile([128, LC, BS], F32)
                    b_inner = contextlib.ExitStack()
                    wfc_pool = b_inner.enter_context(
                        tc.tile_pool(name="wfc", bufs=2)
                    )
                    ps_b = b_inner.enter_context(
                        tc.tile_pool(name="ps_b", bufs=1, space="PSUM")
                    )
                    bbn_ps = [
                        ps_b.tile([128, BS], F32, name=f"bps{lc}", tag=f"bps{lc}")
                        for lc in range(LC)
                    ]
                    cbn_ps = [
                        ps_b.tile([128, BS], F32, name=f"cps{lc}", tag=f"cps{lc}")
                        for lc in range(LC)
                    ]
                    for h in range(HC):
                        w1b = wfc_pool.tile([128, L], F32, tag="w1")
                        w2b = wfc_pool.tile([128, L], F32, tag="w2")
                        nc.sync.dma_start(w1b[:], w_fc1[h * 128 : (h + 1) * 128, :])
                        nc.sync.dma_start(w2b[:], w_fc2[h * 128 : (h + 1) * 128, :])
                        for lc in range(LC):
                            nc.tensor.matmul(
                                bbn_ps[lc][:],
                                w1b[:, lc * 128 : (lc + 1) * 128],
                                featT[:, h, :],
                                start=(h == 0),
                                stop=(h == HC - 1),
                            )
                            nc.tensor.matmul(
                                cbn_ps[lc][:],
                                w2b[:, lc * 128 : (lc + 1) * 128],
                                featT[:, h, :],
                                start=(h == 0),
                                stop=(h == HC - 1),
                            )
                    for lc in range(LC):
                        nc.scalar.activation(
                            out=bbnT[:, lc, :],
                            in_=bbn_ps[lc][:],
                            func=AF.Identity,
                            bias=bfc1_sb[:, lc : lc + 1],
                            scale=1.0,
                        )
                        nc.scalar.activation(
                            out=cbnT_stage[:, lc, :],
                            in_=cbn_ps[lc][:],
                            func=AF.Identity,
                            bias=bfc2_sb[:, lc : lc + 1],
                            scale=1.0,
                        )
                    b_inner.close()
                    # cbn rows -> DRAM -> AllGather (early; hidden by phase C)
                    ps_tr = b_inner.enter_context(
                        tc.tile_pool(name="ps_tr", bufs=4, space="PSUM")
                    )
                    cbnr = cbns.tile([128, BT, L], DTB)
                    for bt in range(BT):
                        for lc in range(LC):
                            tp = ps_tr.tile([128, 128], F32, tag="tp")
                            nc.tensor.transpose(
                                tp[:],
                                cbnT_stage[:, lc, bt * 128 : (bt + 1) * 128],
                                ident_sb[:],
                            )
                            nc.vector.tensor_copy(
                                cbnr[:, bt, lc * 128 : (lc + 1) * 128], tp[:]
                            )
                    nc.sync.dma_start(
                        cbnag_in.ap().rearrange("(t p) l -> p t l", p=128), cbnr[:]
                    )
                    nc.gpsimd.collective_compute(
                        "AllGather",
                        ALU.bypass,
                        replica_groups=rg,
                        ins=[cbnag_in.ap().opt()],
                        outs=[cbnag_out.ap().opt()],
                    )
                    b_inner.close()

                # ---------- phases C+D: distance/argmin/one-hot/gather/vn ---
                with (
                    tc.tile_pool(name="gat", bufs=1) as gatp,
                    tc.tile_pool(name="small_c", bufs=2 * BT) as smallp,
                    tc.tile_pool(name="csqp", bufs=1) as csqp,
                    tc.tile_pool(name="c2", bufs=2) as c2pool,
                    tc.tile_pool(name="sbuf_s", bufs=2) as spool,
                    tc.tile_pool(name="ps_c", bufs=6, space="PSUM") as ps_c,
                    tc.tile_pool(name="ps_d", bufs=2, space="PSUM") as ps_d,
                ):
                    csq_bc = csqp.tile([128, K], F32)
                    nc.scalar.dma_start(csq_bc[:], bcast_row(csq, K))

                    for p, bts in PAIRS:
                        pw = 128 * len(bts)
                        s_bufs = {
                            t: spool.tile([128, K], F32, name=f"sbuf{t}", tag="sbuf")
                            for t in bts
                        }
                        for kc in range(KC):
                            c2b = c2pool.tile([128, LC, 512], F32, tag="c2b")
                            nc.sync.dma_start(
                                c2b[:],
                                cT2.ap()[:, kc * 512 : (kc + 1) * 512].rearrange(
                                    "(c p) k -> p c k", p=128
                                ),
                            )
                            for t in bts:
                                s_ps = ps_c.tile([128, 512], F32, tag="sps")
                                for lc in range(LC):
                                    nc.tensor.matmul(
                                        s_ps[:],
                                        bbnT[:, lc, t * 128 : (t + 1) * 128],
                                        c2b[:, lc, :],
                                        start=(lc == 0),
                                        stop=(lc == LC - 1),
                                    )
                                # score = 2*bbn.c - ||c||^2 (argmax==argmin d)
                                nc.vector.scalar_tensor_tensor(
                                    out=s_bufs[t][:, kc * 512 : (kc + 1) * 512],
                                    in0=s_ps[:],
                                    scalar=1.0,
                                    in1=csq_bc[:, kc * 512 : (kc + 1) * 512],
                                    op0=ALU.mult,
                                    op1=ALU.subtract,
                                )
                        # argmin (one-hot deferred until after the vn AG)
                        idx_fs = {}
                        for t in bts:
                            mx8 = smallp.tile([128, 8], F32, tag="mx8")
                            ix8 = smallp.tile([128, 8], mybir.dt.uint32, tag="ix8")
                            nc.vector.max(mx8[:], s_bufs[t][:])
                            nc.vector.max_index(ix8[:], mx8[:], s_bufs[t][:])
                            nc.vector.tensor_copy(idx_u32[:, t : t + 1], ix8[:, 0:1])
                            idx_f = smallp.tile(
                                [128, 1], F32, name=f"idxf{t}", tag=f"idxf{t % 2}"
                            )
                            nc.vector.tensor_copy(idx_f[:], ix8[:, 0:1])
                            idx_fs[t] = idx_f

                        # per-tile gather + normalize + transpose -> vnT
                        off = bts[0] * 128
                        for t in bts:
                            toff = t * 128
                            idx16 = smallp.tile(
                                [128, 1], mybir.dt.int16, name=f"idx16_{t}",
                                tag=f"idx16_{t % 2}",
                            )
                            nc.vector.tensor_copy(idx16[:], idx_u32[:, t : t + 1])
                            nc.gpsimd.dma_start(
                                idx_d.ap()[toff : toff + 128].rearrange(
                                    "(t p) -> p t", p=128
                                ),
                                idx16[:],
                            )
                            idx_w = gatp.tile(
                                [128, 8], mybir.dt.int16, name=f"idxw{t}",
                                tag=f"idxw{t % 2}",
                            )
                            for r in range(8):
                                nc.gpsimd.dma_start(
                                    idx_w[16 * r : 16 * (r + 1), :],
                                    idx_d.ap()[toff : toff + 128].rearrange(
                                        "(s q) -> q s", q=16
                                    ),
                                )
                            quant = gatp.tile(
                                [128, 1, L], F32, name=f"qt{t}", tag=f"qt{t % 2}"
                            )
                            nc.gpsimd.dma_gather(
                                out_ap=quant[:],
                                in_ap=ctx.ap(),
                                idxs_ap=idx_w[:],
                                num_idxs=128,
                                num_idxs_reg=128,
                                elem_size=L,
                            )
                            sqtmp = gatp.tile(
                                [128, L], F32, name=f"sq{t}", tag=f"sq{t % 2}"
                            )
                            ss = smallp.tile([128, 1], F32, tag="ssn")
                            nc.scalar.activation(
                                out=sqtmp[:],
                                in_=quant[:, 0, :],
                                func=AF.Square,
                                accum_out=ss[:],
                            )
                            nc.scalar.activation(
                                out=ss[:], in_=ss[:], func=AF.Sqrt, bias=eps_sb[:, 0:1]
                            )
                            nc.vector.reciprocal(out=ss[:], in_=ss[:])
                            nc.vector.tensor_scalar(
                                out=quant[:, 0, :],
                                in0=quant[:, 0, :],
                                scalar1=ss[:],
                                scalar2=None,
                                op0=ALU.mult,
                            )
                            for lc in range(LC):
                                tp = ps_d.tile([128, 128], F32, tag="tp")
                                nc.tensor.transpose(
                                    tp[:],
                                    quant[:, 0, lc * 128 : (lc + 1) * 128],
                                    ident_sb[:],
                                )
                                nc.vector.tensor_copy(
                                    vnT[:, lc, t * 128 : (t + 1) * 128], tp[:]
                                )
                            # one-hot context_ind rows (off critical path)
                            KH = K // 2
                            for hh in range(2):
                                oh = ohpool.tile(
                                    [128, KH], F32, name=f"oh{t}_{hh}", tag="oh"
                                )
                                nc.scalar.dma_start(
                                    oh[:],
                                    iota.ap()[hh * KH : (hh + 1) * KH]
                                    .rearrange("(q j) -> q j", q=1)
                                    .to_broadcast([128, KH]),
                                )
                                nc.vector.tensor_scalar(
                                    out=oh[:],
                                    in0=oh[:],
                                    scalar1=idx_fs[t][:],
                                    scalar2=None,
                                    op0=ALU.is_equal,
                                )
                                nc.scalar.dma_start(
                                    cind_o[
                                        t * 128 : (t + 1) * 128,
                                        hh * KH : (hh + 1) * KH,
                                    ],
                                    oh[:],
                                )
                        nc.gpsimd.dma_start(
                            vnag_ins[p]
                            .ap()
                            .rearrange("(c q) b -> q c b", q=128),
                            vnT[:, :, off : off + pw],
                        )
                        nc.gpsimd.collective_compute(
                            "AllGather",
                            ALU.bypass,
                            replica_groups=rg,
                            ins=[vnag_ins[p].ap().opt()],
                            outs=[vnag_outs[p].ap().opt()],
                        )


            # ---------- phases E+F: adjacency + GCN -------------------------
            with tc.tile_pool(name="adj", bufs=1) as adjp:
                adj_sb = adjp.tile([128, BT, B], F32)
                dinv_sb = tlay.tile([128, JC], F32)
                with (
                    tc.tile_pool(name="small_e", bufs=2 * BT) as small_e,
                    tc.tile_pool(name="ps_e", bufs=3, space="PSUM") as ps_e,
                    tc.tile_pool(name="ps_dv", bufs=2, space="PSUM") as ps_dv,
                ):
                    NP = len(PAIRS)
                    rss = [
                        small_e.tile(
                            [128, N_CORES * NP], F32, name=f"rs{bt}", tag=f"rs{bt}"
                        )
                        for bt in range(BT)
                    ]
                    # partial row-sums of vn (for the closed-form column sums)
                    spart = small_e.tile([128, LC, NP * N_CORES], F32, name="spart")
                    s_col = small_e.tile([128, LC], F32, name="s_col")
                    s_col16 = small_e.tile([128, LC], DTB, name="s_col16")
                    vnag_sbs = {}
                    vnag_stack = contextlib.ExitStack()
                    for p, bts in PAIRS:
                        pw = 128 * len(bts)
                        off = bts[0] * 128
                        vnagp = vnag_stack.enter_context(
                            tc.tile_pool(name=f"vnag{p}", bufs=1)
                        )
                        vsb = vnagp.tile(
                            [128, N_CORES * LC, pw],
                            DTB,
                            name=f"vnag{p}",
                            tag=f"vg{p}",
                        )
                        nc.scalar.dma_start(
                            vsb[:],
                            vnag_outs[p].ap().rearrange("(c q) b -> q c b", q=128),
                        )
                        vnag_sbs[p] = vsb
                        for c in range(N_CORES):
                            for lc in range(LC):
                                nc.vector.reduce_sum(
                                    spart[:, lc, p * N_CORES + c : p * N_CORES + c + 1],
                                    vsb[:, c * LC + lc, :],
                                    axis=mybir.AxisListType.X,
                                )
                        for bt in range(BT):
                            for c in range(N_CORES):
                                a_ps = ps_e.tile([128, pw], F32, tag="aps")
                                for lc in range(LC):
                                    nc.tensor.matmul(
                                        a_ps[:],
                                        vnT[:, lc, bt * 128 : (bt + 1) * 128],
                                        vnag_sbs[p][:, c * LC + lc, :],
                                        start=(lc == 0),
                                        stop=(lc == LC - 1),
                                    )
                                nc.scalar.activation(
                                    out=adj_sb[
                                        :, bt, c * BS + off : c * BS + off + pw
                                    ],
                                    in_=a_ps[:],
                                    func=AF.Identity,
                                    bias=half_sb[:, 0:1],
                                    scale=0.5,
                                    accum_out=rss[bt][
                                        :, p * N_CORES + c : p * N_CORES + c + 1
                                    ],
                                )
                    # dinv for our rows (exact row sums, matches reference)
                    for bt in range(BT):
                        rsum = small_e.tile([128, 1], F32, tag="rsum")
                        nc.vector.reduce_sum(
                            rsum[:], rss[bt][:], axis=mybir.AxisListType.X
                        )
                        nc.scalar.activation(
                            out=rsum[:], in_=rsum[:], func=AF.Sqrt, bias=eps_sb[:, 0:1]
                        )
                        nc.vector.reciprocal(out=rsum[:], in_=rsum[:])
                        nc.vector.tensor_copy(dinv_bt[:, bt : bt + 1], rsum[:])
                        nc.scalar.dma_start(
                            adj_o[bt * 128 : (bt + 1) * 128, :], adj_sb[:, bt, :]
                        )
                    # dinv for all columns, closed form:
                    # colsum_j = 0.5*(vn_j . S) + 0.5*B  (S = sum of all vn rows)
                    nc.vector.reduce_sum(
                        s_col[:], spart[:], axis=mybir.AxisListType.X
                    )
                    nc.vector.tensor_copy(s_col16[:], s_col[:])
                    for m in range(JC):
                        c, t = m // BT, m % BT
                        p, o2 = t // 2, (t % 2) * 128
                        d_ps = ps_dv.tile([128, 1], F32, tag="dv")
                        for lc in range(LC):
                            nc.tensor.matmul(
                                d_ps[:],
                                vnag_sbs[p][:, c * LC + lc, o2 : o2 + 128],
                                s_col16[:, lc : lc + 1],
                                start=(lc == 0),
                                stop=(lc == LC - 1),
                            )
                        nc.scalar.activation(
                            out=dinv_sb[:, m : m + 1],
                            in_=d_ps[:],
                            func=AF.Sqrt,
                            bias=halfb_sb[:, 0:1],
                            scale=0.5,
                        )
                    nc.vector.reciprocal(out=dinv_sb[:], in_=dinv_sb[:])
                    vnag_stack.close()
                    # bbn/feat row-major outputs: fills the dinv-AllGather gap
                    with tc.tile_pool(name="stg", bufs=1) as stg:
                        for bt in range(BT):
                            bstg = stg.tile([128, L], F32, tag="bstg")
                            for lc in range(LC):
                                tp = ps_e.tile([128, 128], F32, tag="tp")
                                nc.tensor.transpose(
                                    tp[:],
                                    bbnT[:, lc, bt * 128 : (bt + 1) * 128],
                                    ident_sb[:],
                                )
                                nc.vector.tensor_copy(
                                    bstg[:, lc * 128 : (lc + 1) * 128], tp[:]
                                )
                            nc.scalar.dma_start(
                                bbn_o[bt * 128 : (bt + 1) * 128, :], bstg[:]
                            )
                        for bt in range(BT):
                            fstg = stg.tile([128, H], F32, tag="fstg")
                            for h in range(HC):
                                tp = ps_e.tile([128, 128], F32, tag="tp")
                                nc.tensor.transpose(
                                    tp[:],
                                    featT[:, h, bt * 128 : (bt + 1) * 128],
                                    ident_sb[:],
                                )
                                nc.vector.tensor_copy(
                                    fstg[:, h * 128 : (h + 1) * 128], tp[:]
                                )
                            nc.scalar.dma_start(
                                feat_o[bt * 128 : (bt + 1) * 128, :], fstg[:]
                            )

                # ---- GCN layer 1 (normalization folded):
                # latpre = dinv_i * sum_j adj[i,j] * (dinv_j * cbn[j])
                with (
                    tc.tile_pool(name="cbnagp", bufs=4) as cbnagp,
                    tc.tile_pool(name="natp", bufs=32) as natp,
                    tc.tile_pool(name="latp", bufs=1) as latp,
                ):
                    f_inner = contextlib.ExitStack()
                    ps_lp = f_inner.enter_context(
                        tc.tile_pool(name="ps_lp", bufs=1, space="PSUM")
                    )
                    ps_tp2 = f_inner.enter_context(
                        tc.tile_pool(name="ps_tp2", bufs=4, space="PSUM")
                    )
                    latpre = latp.tile([128, BT, L], F32)
                    lp_ps = [
                        ps_lp.tile([128, L], F32, name=f"lpps{bt}", tag=f"lpps{bt}")
                        for bt in range(BT)
                    ]
                    for m in range(JC):
                        cb = cbnagp.tile([128, L], DTB, tag="cb")
                        nc.sync.dma_start(cb[:], cbnag_out[m * 128 : (m + 1) * 128, :])
                        nc.vector.tensor_scalar(
                            out=cb[:],
                            in0=cb[:],
                            scalar1=dinv_sb[:, m : m + 1],
                            scalar2=None,
                            op0=ALU.mult,
                        )
                        naTs = []
                        for bt in range(BT):
                            tp = ps_tp2.tile([128, 128], F32, tag="tp")
                            nc.tensor.transpose(
                                tp[:],
                                adj_sb[:, bt, m * 128 : (m + 1) * 128],
                                ident_sb[:],
                            )
                            naT = natp.tile([128, 128], DTB, tag="naT")
                            nc.vector.tensor_copy(naT[:], tp[:])
                            naTs.append(naT)
                        for bt in range(BT):
                            nc.tensor.matmul(
                                lp_ps[bt][:],
                                naTs[bt][:],
                                cb[:],
                                start=(m == 0),
                                stop=(m == JC - 1),
                            )
                    for bt in range(BT):
                        nc.scalar.activation(
                            out=latpre[:, bt, :],
                            in_=lp_ps[bt][:],
                            func=AF.Copy,
                            scale=dinv_bt[:, bt : bt + 1],
                        )

                    # latpreT + zT = W_gcn.T @ latpreT, sigmoid -> latT
                    latpreT = latp.tile([128, LC, BS], DTB)
                    for bt in range(BT):
                        for lc in range(LC):
                            tp = ps_tp2.tile([128, 128], F32, tag="tp")
                            nc.tensor.transpose(
                                tp[:],
                                latpre[:, bt, lc * 128 : (lc + 1) * 128],
                                ident_sb[:],
                            )
                            nc.vector.tensor_copy(
                                latpreT[:, lc, bt * 128 : (bt + 1) * 128], tp[:]
                            )
                    f_inner.close()
                    with (
                        tc.tile_pool(name="wgcnp", bufs=1) as wgcnp,
                        tc.tile_pool(name="ps_z", bufs=2, space="PSUM") as ps_z,
                    ):
                        wgcn_sb = wgcnp.tile([128, LC, L], DTB)
                        nc.sync.dma_start(
                            wgcn_sb[:], w_gcn.ap().rearrange("(c p) l -> p c l", p=128)
                        )
                        for gc in range(LC):
                            z_ps = ps_z.tile([128, BS], F32, tag="z")
                            for lc in range(LC):
                                nc.tensor.matmul(
                                    z_ps[:],
                                    wgcn_sb[:, lc, gc * 128 : (gc + 1) * 128],
                                    latpreT[:, lc, :],
                                    start=(lc == 0),
                                    stop=(lc == LC - 1),
                                )
                            nc.scalar.activation(
                                out=latT[:, gc, :],
                                in_=z_ps[:],
                                func=AF.Sigmoid,
                                bias=bgcn_sb[:, gc : gc + 1],
                                scale=1.0,
                            )

            # ---------- phase G: decoder ------------------------------------
            with (
                tc.tile_pool(name="wdecp", bufs=2) as wdecp,
                tc.tile_pool(name="d1p", bufs=1) as d1p,
                tc.tile_pool(name="b2p", bufs=1) as b2p,
                tc.tile_pool(name="dstage", bufs=3) as dstage,
                tc.tile_pool(name="ps_d1", bufs=2, space="PSUM") as ps_d1,
                tc.tile_pool(name="ps_o", bufs=4, space="PSUM") as ps_o,
            ):
                wdec1_sb = wdecp.tile([128, LC, H], DTB, tag="w1")
                nc.sync.dma_start(
                    wdec1_sb[:], w_dec1.ap().rearrange("(c p) h -> p c h", p=128)
                )
                d1T = d1p.tile([128, HC, BS], DTB)
                for hc in range(HC):
                    d_ps = ps_d1.tile([128, BS], F32, tag="d1")
                    for gc in range(LC):
                        nc.tensor.matmul(
                            d_ps[:],
                            wdec1_sb[:, gc, hc * 128 : (hc + 1) * 128],
                            latT[:, gc, :],
                            start=(gc == 0),
                            stop=(gc == LC - 1),
                        )
                    nc.scalar.activation(
                        out=d1T[:, hc, :],
                        in_=d_ps[:],
                        func=AF.Relu,
                        bias=bdec1_sb[:, hc : hc + 1],
                        scale=1.0,
                    )
                b2_bc = b2p.tile([128, Fd], F32)
                nc.sync.dma_start(b2_bc[:], bcast_row(b_dec2, Fd))
                for fc in range(Fd // 512):
                    w2blk = wdecp.tile([128, HC, 512], DTB, tag="w2")
                    nc.sync.dma_start(
                        w2blk[:],
                        w_dec2.ap()[:, fc * 512 : (fc + 1) * 512].rearrange(
                            "(c p) f -> p c f", p=128
                        ),
                    )
                    for bt in range(BT):
                        o_ps = ps_o.tile([128, 512], F32, tag="ops")
                        for hc in range(HC):
                            nc.tensor.matmul(
                                o_ps[:],
                                d1T[:, hc, bt * 128 : (bt + 1) * 128],
                                w2blk[:, hc, :],
                                start=(hc == 0),
                                stop=(hc == HC - 1),
                            )
                        dec_sb = dstage.tile([128, 512], F32, tag="dec")
                        nc.vector.tensor_tensor(
                            out=dec_sb[:],
                            in0=o_ps[:],
                            in1=b2_bc[:, fc * 512 : (fc + 1) * 512],
                            op=ALU.add,
                        )
                        nc.sync.dma_start(
                            decoded_o[
                                bt * 128 : (bt + 1) * 128, fc * 512 : (fc + 1) * 512
                            ],
                            dec_sb[:],
                        )

    nc.compile()
    return nc


def _wcast(w):
    w = np.ascontiguousarray(np.asarray(w, np.float32))
    if BF16_BACKEND:
        import ml_dtypes

        return w.astype(ml_dtypes.bfloat16)
    return w


def prep_inputs(inputs, B=4096, Fd=4096, H=1024, L=512, K=8192):
    """Host-side shard + transpose. Returns per-core input maps."""
    BS = B // N_CORES
    x = np.asarray(inputs["x"], np.float32)
    context = np.ascontiguousarray(np.asarray(inputs["context"], np.float32))
    cT2 = np.ascontiguousarray(context.T * 2.0)
    csq = np.sum(context * context, axis=-1).astype(np.float32)
    iota = np.arange(K, dtype=np.float32)
    ident = np.eye(128, dtype=np.float32)
    shared = {
        "w_enc": np.ascontiguousarray(np.asarray(inputs["W_enc"], np.float32)),
        "b_enc": np.asarray(inputs["b_enc"], np.float32),
        "w_fc1": np.ascontiguousarray(np.asarray(inputs["W_fc1"], np.float32)),
        "b_fc1": np.asarray(inputs["b_fc1"], np.float32),
        "w_fc2": np.ascontiguousarray(np.asarray(inputs["W_fc2"], np.float32)),
        "b_fc2": np.asarray(inputs["b_fc2"], np.float32),
        "cT2": cT2,
        "csq": csq,
        "ctx": context,
        "iota": iota,
        "ident": ident,
        "w_gcn": _wcast(inputs["W_gcn"]),
        "b_gcn": np.asarray(inputs["b_gcn"], np.float32),
        "w_dec1": _wcast(inputs["W_dec1"]),
        "b_dec1": np.asarray(inputs["b_dec1"], np.float32),
        "w_dec2": _wcast(inputs["W_dec2"]),
        "b_dec2": np.asarray(inputs["b_dec2"], np.float32),
    }
    in_maps = []
    for c in range(N_CORES):
        m = dict(shared)
        m["xT"] = np.ascontiguousarray(x[c * BS : (c + 1) * BS, :].T)
        in_maps.append(m)
    return in_maps


def collect_outputs(results):
    decoded = np.concatenate([r["decoded"] for r in results], axis=0)
    bbn = np.concatenate([r["bbn"] for r in results], axis=0)
    cind = np.concatenate([r["context_ind"] for r in results], axis=0)
    feat = np.concatenate([r["feat"] for r in results], axis=0)
    adj = np.concatenate([r["adj"] for r in results], axis=0)
    return decoded, bbn, cind, feat, adj


_NC_CACHE = {}


def kernel(**inputs):
    B, Fd, H, L, K = 4096, 4096, 1024, 512, 8192
    _install_ntff_hook()
    from concourse.bass_utils import run_bass_kernel_spmd

    key = (B, Fd, H, L, K)
    if key not in _NC_CACHE:
        _NC_CACHE[key] = build(B, Fd, H, L, K)
    nc = _NC_CACHE[key]
    in_maps = prep_inputs(inputs, B, Fd, H, L, K)
    res = run_bass_kernel_spmd(nc, in_maps, core_ids=list(range(N_CORES)))
    return collect_outputs(res.results)
